# revision 3
# baseline (speedup 1.0000x reference)
"""Trainium2 Bass kernel for AttentionWithSharedWeights (LoRA attention, GQA, RoPE).

Sharding over 8 NeuronCores: batch (4) x head-group (2).  Each core computes
8 Q heads / 2 KV heads of one batch and a partial (head-sliced) output
projection; the host sums the two partials per batch.

All matmuls run in float32r (full-rate fp32 on the PE array).
"""

import numpy as np

B, S, DIM = 4, 2048, 2048
NH, NKV, HD = 16, 4, 128
LR = 16          # lora rank
SC = 512         # sequence chunk
NSC = S // SC    # 4
NKT = S // HD    # 16 k-tiles
HPC = NH // 2    # 8 q heads per core
KVPC = NKV // 2  # 2 kv heads per core
FQ = HPC * HD    # 1024 q features per core
FKV = KVPC * HD  # 256 kv features per core
SCALE = 1.0 / float(np.sqrt(HD))

_cache = {}


def _build_program():
    import concourse.mybir as mybir
    import concourse.tile as tile
    from concourse import bacc

    f32 = mybir.dt.float32
    f32r = mybir.dt.float32r
    Exp = mybir.ActivationFunctionType.Exp

    nc = bacc.Bacc()

    # ---- DRAM parameters (per-core views, host-prepared layouts) ----
    xt_d = nc.declare_dram_parameter("xt", [DIM, S], f32r, isOutput=False)
    wq_d = nc.declare_dram_parameter("wq", [DIM, FQ], f32r, isOutput=False)
    wk_d = nc.declare_dram_parameter("wk", [DIM, FKV], f32r, isOutput=False)
    wv_d = nc.declare_dram_parameter("wv", [DIM, FKV], f32r, isOutput=False)
    wo_d = nc.declare_dram_parameter("wo", [FQ, DIM], f32r, isOutput=False)
    a6_d = nc.declare_dram_parameter("a6", [DIM, 3 * LR], f32r, isOutput=False)
    bq_d = nc.declare_dram_parameter("bq", [LR, FQ], f32r, isOutput=False)
    bk_d = nc.declare_dram_parameter("bk", [LR, FKV], f32r, isOutput=False)
    bv_d = nc.declare_dram_parameter("bv", [LR, FKV], f32r, isOutput=False)
    ao_d = nc.declare_dram_parameter("ao", [FQ, LR], f32r, isOutput=False)
    bo_d = nc.declare_dram_parameter("bo", [LR, DIM], f32r, isOutput=False)
    cs_d = nc.declare_dram_parameter("cs", [HD, S], f32, isOutput=False)
    sn_d = nc.declare_dram_parameter("sn", [HD, S], f32, isOutput=False)
    rt_d = nc.declare_dram_parameter("rt", [HD, HD], f32r, isOutput=False)
    on_d = nc.declare_dram_parameter("on", [HD, HD], f32r, isOutput=False)
    mk_d = nc.declare_dram_parameter("mk", [HD, 4, SC], f32, isOutput=False)
    y_d = nc.declare_dram_parameter("y", [S, DIM], f32, isOutput=True)

    # internal spills
    qt_d = nc.dram_tensor("qt_spill", [FQ, S], f32r)
    ot_d = nc.dram_tensor("ot_spill", [FQ, S], f32r)

    with tile.TileContext(nc) as tc:
        # K/V stay in SBUF across phases A and B
        with tc.tile_pool(name="kvres", bufs=1) as kvres:
            kt_sb = kvres.tile([HD, KVPC, S], f32r)       # K feat-major, roped
            v_sb = kvres.tile([HD, NKT, FKV], f32r)       # V token-major

            # ---------------- Phase A: projections + RoPE ----------------
            with tc.tile_pool(name="pa_w", bufs=1) as pw, \
                 tc.tile_pool(name="pa_x", bufs=2) as px, \
                 tc.tile_pool(name="pa_wq", bufs=2) as pwq, \
                 tc.tile_pool(name="pa_t", bufs=2) as pt, \
                 tc.tile_pool(name="pa_r", bufs=3) as pr, \
                 tc.tile_pool(name="pa_ps", bufs=2, space="PSUM") as pps, \
                 tc.tile_pool(name="pa_rot", bufs=2, space="PSUM") as prot, \
                 tc.tile_pool(name="pa_vps", bufs=2, space="PSUM") as pvps, \
                 tc.tile_pool(name="pa_tps", bufs=1, space="PSUM") as ptps:

                wk_sb = pw.tile([HD, NKT, FKV], f32r)
                wv_sb = pw.tile([HD, NKT, FKV], f32r)
                a6_sb = pw.tile([HD, NKT, 3 * LR], f32r)
                bq_sb = pw.tile([LR, FQ], f32r)
                bk_sb = pw.tile([LR, FKV], f32r)
                bv_sb = pw.tile([LR, FKV], f32r)
                cs_sb = pw.tile([HD, S], f32)
                sn_sb = pw.tile([HD, S], f32)
                rt_sb = pw.tile([HD, HD], f32r)

                nc.sync.dma_start(wk_sb[:], wk_d[:].rearrange("(k p) f -> p k f", p=HD))
                nc.sync.dma_start(wv_sb[:], wv_d[:].rearrange("(k p) f -> p k f", p=HD))
                nc.sync.dma_start(a6_sb[:], a6_d[:].rearrange("(k p) f -> p k f", p=HD))
                nc.sync.dma_start(bq_sb[:], bq_d[:])
                nc.sync.dma_start(bk_sb[:], bk_d[:])
                nc.sync.dma_start(bv_sb[:], bv_d[:])
                nc.sync.dma_start(cs_sb[:], cs_d[:])
                nc.sync.dma_start(sn_sb[:], sn_d[:])
                nc.sync.dma_start(rt_sb[:], rt_d[:])

                def rope_block(raw_ps, fpool, cs_sl, sn_sl, out_ap):
                    """raw_ps: PSUM [128, SC] pre-rope; writes roped f32r to out_ap."""
                    raw = fpool.tile([HD, SC], f32r, tag="rope_raw")
                    nc.any.tensor_copy(out=raw[:], in_=raw_ps[:])
                    rot_ps = prot.tile([HD, SC], f32)
                    nc.tensor.matmul(rot_ps[:], rt_sb[:], raw[:],
                                     start=True, stop=True)
                    tmp = fpool.tile([HD, SC], f32, tag="rope_tmp")
                    nc.vector.tensor_mul(tmp[:], raw[:].bitcast(f32), cs_sl)
                    e1 = fpool.tile([HD, SC], f32, tag="rope_e1")
                    nc.vector.tensor_mul(e1[:], rot_ps[:], sn_sl)
                    nc.vector.tensor_add(out_ap, tmp[:], e1[:])

                for sc in range(NSC):
                    ssl = slice(sc * SC, (sc + 1) * SC)
                    xc = px.tile([HD, NKT, SC], f32r, tag="xc")
                    for kt in range(NKT):
                        nc.sync.dma_start(
                            xc[:, kt, :], xt_d[kt * HD:(kt + 1) * HD, ssl])

                    # lora t = A @ x^T  (three rank-16 groups)
                    tq = pt.tile([LR, SC], f32r, tag="tq")
                    tk = pt.tile([LR, SC], f32r, tag="tk")
                    tv = pt.tile([LR, SC], f32r, tag="tv")
                    for i, tdst in enumerate((tq, tk, tv)):
                        t_ps = ptps.tile([LR, SC], f32, tag="t_ps")
                        for kt in range(NKT):
                            nc.tensor.matmul(
                                t_ps[:], a6_sb[:, kt, i * LR:(i + 1) * LR],
                                xc[:, kt, :],
                                start=(kt == 0), stop=(kt == NKT - 1))
                        nc.any.tensor_copy(out=tdst[:], in_=t_ps[:])

                    # Q projection + rope -> spill to DRAM
                    for ft in range(HPC):
                        fsl = slice(ft * HD, (ft + 1) * HD)
                        wqt = pwq.tile([HD, NKT, HD], f32r, tag="wqt")
                        nc.sync.dma_start(
                            wqt[:], wq_d[:, fsl].rearrange("(k p) f -> p k f", p=HD))
                        q_ps = pps.tile([HD, SC], f32, tag="qk_ps")
                        for kt in range(NKT):
                            nc.tensor.matmul(q_ps[:], wqt[:, kt, :], xc[:, kt, :],
                                             start=(kt == 0), stop=False)
                        nc.tensor.matmul(q_ps[:], bq_sb[:, fsl], tq[:],
                                         start=False, stop=True)
                        qfin = pr.tile([HD, SC], f32r, tag="qfin")
                        rope_block(q_ps, pr, cs_sb[:, ssl], sn_sb[:, ssl], qfin[:])
                        nc.sync.dma_start(qt_d[fsl, ssl], qfin[:])

                    # K projection + rope -> resident SBUF
                    for ft in range(KVPC):
                        fsl = slice(ft * HD, (ft + 1) * HD)
                        k_ps = pps.tile([HD, SC], f32, tag="qk_ps")
                        for kt in range(NKT):
                            nc.tensor.matmul(k_ps[:], wk_sb[:, kt, fsl], xc[:, kt, :],
                                             start=(kt == 0), stop=False)
                        nc.tensor.matmul(k_ps[:], bk_sb[:, fsl], tk[:],
                                         start=False, stop=True)
                        rope_block(k_ps, pr, cs_sb[:, ssl], sn_sb[:, ssl],
                                   kt_sb[:, ft, ssl])

                    # V projection, token-major -> resident SBUF
                    for st in range(SC // HD):
                        tsl = slice(st * HD, (st + 1) * HD)
                        v_ps = pvps.tile([HD, FKV], f32, tag="v_ps")
                        for kt in range(NKT):
                            nc.tensor.matmul(v_ps[:], xc[:, kt, tsl], wv_sb[:, kt, :],
                                             start=(kt == 0), stop=False)
                        nc.tensor.matmul(v_ps[:], tv[:, tsl], bv_sb[:],
                                         start=False, stop=True)
                        nc.any.tensor_copy(
                            out=v_sb[:, sc * (SC // HD) + st, :], in_=v_ps[:])

            tc.strict_bb_all_engine_barrier()

            # ---------------- Phase B: causal attention ----------------
            with tc.tile_pool(name="pb_c", bufs=1) as pbc, \
                 tc.tile_pool(name="pb_q", bufs=2) as pbq, \
                 tc.tile_pool(name="pb_e", bufs=4) as pbe, \
                 tc.tile_pool(name="pb_o", bufs=2) as pbo, \
                 tc.tile_pool(name="pb_sps", bufs=2, space="PSUM") as sps, \
                 tc.tile_pool(name="pb_ops", bufs=2, space="PSUM") as ops, \
                 tc.tile_pool(name="pb_bps", bufs=2, space="PSUM") as bps:

                on_sb = pbc.tile([HD, HD], f32r)
                mk_sb = pbc.tile([HD, 4, SC], f32)
                nc.sync.dma_start(on_sb[:], on_d[:])
                nc.sync.dma_start(mk_sb[:], mk_d[:])

                for h in range(HPC):
                    kv = h // (HPC // KVPC)
                    for qc in range(NSC):
                        qsl = slice(qc * SC, (qc + 1) * SC)
                        qt = pbq.tile([HD, SC], f32r, tag="qt")
                        nc.sync.dma_start(
                            qt[:], qt_d[h * HD:(h + 1) * HD, qsl])
                        ot_ps = ops.tile([HD, SC], f32, tag="ot_ps")
                        esum = pbe.tile([HD, SC], f32r, tag="esum")
                        nkt = 4 * qc + 4
                        for kt in range(nkt):
                            s_ps = sps.tile([HD, SC], f32, tag="s_ps")
                            nc.tensor.matmul(
                                s_ps[:], kt_sb[:, kv, kt * HD:(kt + 1) * HD],
                                qt[:], start=True, stop=True)
                            e = pbe.tile([HD, SC], f32r, tag="e")
                            nc.scalar.activation(e[:], s_ps[:], Exp, scale=SCALE)
                            if kt >= 4 * qc:
                                em = pbe.tile([HD, SC], f32r, tag="em")
                                nc.vector.tensor_mul(
                                    em[:], e[:].bitcast(f32),
                                    mk_sb[:, kt - 4 * qc, :])
                                e = em
                            nc.tensor.matmul(
                                ot_ps[:], v_sb[:, kt, kv * HD:(kv + 1) * HD],
                                e[:], start=(kt == 0), stop=(kt == nkt - 1),
                                skip_group_check=True)
                            if kt == 0:
                                nc.vector.tensor_copy(esum[:], e[:].bitcast(f32))
                            else:
                                nc.vector.tensor_add(
                                    esum[:], esum[:].bitcast(f32),
                                    e[:].bitcast(f32))
                        bc_ps = bps.tile([HD, SC], f32, tag="bc_ps")
                        nc.tensor.matmul(bc_ps[:], on_sb[:], esum[:],
                                         start=True, stop=True)
                        inv = pbo.tile([HD, SC], f32, tag="inv")
                        nc.vector.reciprocal(inv[:], bc_ps[:])
                        ot_sb = pbo.tile([HD, SC], f32r, tag="ot_sb")
                        nc.vector.tensor_mul(ot_sb[:], ot_ps[:], inv[:])
                        nc.sync.dma_start(
                            ot_d[h * HD:(h + 1) * HD, qsl], ot_sb[:])

        tc.strict_bb_all_engine_barrier()

        # ---------------- Phase C: output projection + LoRA ----------------
        with tc.tile_pool(name="pc_w", bufs=1) as pcw, \
             tc.tile_pool(name="pc_o", bufs=2) as pco, \
             tc.tile_pool(name="pc_y", bufs=3) as pcy, \
             tc.tile_pool(name="pc_yps", bufs=4, space="PSUM") as yps, \
             tc.tile_pool(name="pc_tps", bufs=2, space="PSUM") as tps:

            wo_sb = pcw.tile([HD, HPC, DIM], f32r)
            ao_sb = pcw.tile([HD, HPC, LR], f32r)
            bo_sb = pcw.tile([LR, DIM], f32r)
            to_sb = pcw.tile([LR, S], f32r)
            for ft in range(HPC):
                nc.sync.dma_start(
                    wo_sb[:, ft, :], wo_d[ft * HD:(ft + 1) * HD, :])
            nc.sync.dma_start(ao_sb[:], ao_d[:].rearrange("(k p) f -> p k f", p=HD))
            nc.sync.dma_start(bo_sb[:], bo_d[:])

            for qc in range(NSC):
                qsl = slice(qc * SC, (qc + 1) * SC)
                otq = pco.tile([HD, HPC, SC], f32r, tag="otq")
                for ft in range(HPC):
                    nc.sync.dma_start(
                        otq[:, ft, :], ot_d[ft * HD:(ft + 1) * HD, qsl])
                # lora t_o for this chunk
                to_ps = tps.tile([LR, SC], f32, tag="to_ps")
                for ft in range(HPC):
                    nc.tensor.matmul(to_ps[:], ao_sb[:, ft, :], otq[:, ft, :],
                                     start=(ft == 0), stop=(ft == HPC - 1))
                nc.any.tensor_copy(out=to_sb[:, qsl], in_=to_ps[:])
                # main projection
                for qs in range(SC // HD):
                    qt0 = qc * SC + qs * HD
                    for dc in range(DIM // SC):
                        dsl = slice(dc * SC, (dc + 1) * SC)
                        y_ps = yps.tile([HD, SC], f32, tag="y_ps")
                        for ft in range(HPC):
                            nc.tensor.matmul(
                                y_ps[:], otq[:, ft, qs * HD:(qs + 1) * HD],
                                wo_sb[:, ft, dsl],
                                start=(ft == 0), stop=False)
                        nc.tensor.matmul(
                            y_ps[:], to_sb[:, qt0:qt0 + HD], bo_sb[:, dsl],
                            start=False, stop=True)
                        y_sb = pcy.tile([HD, SC], f32, tag="y_sb")
                        nc.any.tensor_copy(out=y_sb[:], in_=y_ps[:])
                        nc.sync.dma_start(y_d[qt0:qt0 + HD, dsl], y_sb[:])

    nc.finalize()
    return nc


def _rope_perm(nheads):
    """Row permutation putting even dims first within each head."""
    idx = []
    for h in range(nheads):
        base = h * HD
        idx.extend(base + 2 * j for j in range(HD // 2))
        idx.extend(base + 2 * j + 1 for j in range(HD // 2))
    return np.array(idx)


def _prepare_in_maps(inputs):
    x = np.ascontiguousarray(np.asarray(inputs["x"], dtype=np.float32))
    fc = np.asarray(inputs["freqs_cos"], dtype=np.float32)
    fs = np.asarray(inputs["freqs_sin"], dtype=np.float32)
    wq = np.asarray(inputs["wq"], dtype=np.float32)
    wk = np.asarray(inputs["wk"], dtype=np.float32)
    wv = np.asarray(inputs["wv"], dtype=np.float32)
    wo = np.asarray(inputs["wo"], dtype=np.float32)
    aq = np.asarray(inputs["aq"], dtype=np.float32)
    bq = np.asarray(inputs["bq"], dtype=np.float32)
    ak = np.asarray(inputs["ak"], dtype=np.float32)
    bk = np.asarray(inputs["bk"], dtype=np.float32)
    av = np.asarray(inputs["av"], dtype=np.float32)
    bv = np.asarray(inputs["bv"], dtype=np.float32)
    ao = np.asarray(inputs["ao"], dtype=np.float32)
    bo = np.asarray(inputs["bo"], dtype=np.float32)

    permQ = _rope_perm(HPC)
    permK = _rope_perm(KVPC)
    a6 = np.ascontiguousarray(np.vstack([aq, ak, av]).T)   # [DIM, 48]
    fcT = np.ascontiguousarray(fc.T)                       # [64, S]
    fsT = np.ascontiguousarray(fs.T)
    cs = np.concatenate([fcT, fcT], axis=0)                # [128, S]
    sn = np.concatenate([fsT, fsT], axis=0)
    rt = np.zeros((HD, HD), np.float32)
    for j in range(HD // 2):
        rt[j, 64 + j] = 1.0      # (R^T)[j, 64+j] = R[64+j, j] = +1
        rt[64 + j, j] = -1.0     # (R^T)[64+j, j] = R[j, 64+j] = -1
    ones = np.ones((HD, HD), np.float32)
    kk = np.arange(HD)[:, None]
    qq = np.arange(SC)[None, :]
    mk = np.stack([(qq >= (128 * r + kk)).astype(np.float32) for r in range(4)],
                  axis=1)                                  # [128, 4, SC]

    xt_cache = {}
    in_maps = []
    for c in range(8):
        b, g = c // 2, c % 2
        if b not in xt_cache:
            xt_cache[b] = np.ascontiguousarray(x[b].T)
        fq = slice(g * FQ, (g + 1) * FQ)
        fkv = slice(g * FKV, (g + 1) * FKV)
        wq_g = wq[fq][permQ]
        wk_g = wk[fkv][permK]
        bq_g = bq[fq][permQ]
        bk_g = bk[fkv][permK]
        in_maps.append({
            "xt": xt_cache[b],
            "wq": np.ascontiguousarray(wq_g.T),
            "wk": np.ascontiguousarray(wk_g.T),
            "wv": np.ascontiguousarray(wv[fkv].T),
            "wo": np.ascontiguousarray(wo[:, fq].T),
            "a6": a6,
            "bq": np.ascontiguousarray(bq_g.T),
            "bk": np.ascontiguousarray(bk_g.T),
            "bv": np.ascontiguousarray(bv[fkv].T),
            "ao": np.ascontiguousarray(ao[:, fq].T),
            "bo": np.ascontiguousarray(bo.T),
            "cs": cs, "sn": sn, "rt": rt, "on": ones, "mk": mk,
        })
    return in_maps


def _get_program():
    if "nc" not in _cache:
        _cache["nc"] = _build_program()
    return _cache["nc"]


def run(inputs, trace=False):
    from concourse import bass_utils
    nc = _get_program()
    in_maps = _prepare_in_maps(inputs)
    res = bass_utils.run_bass_kernel_spmd(
        nc, in_maps, list(range(8)), trace=trace)
    ys = [res.results[c]["y"] for c in range(8)]
    out = np.empty((B, S, DIM), np.float32)
    for b in range(B):
        out[b] = ys[2 * b] + ys[2 * b + 1]
    return out, res


def kernel(**inputs):
    out, _ = run(inputs, trace=False)
    return out


def bench(inputs, iters=20):
    """Time repeated NEFF executions with device-resident inputs.

    Mirrors bass2jax.run_bass_via_pjrt's multi-core path without donation so
    buffers can be reused across calls.  Returns (avg_exec_seconds, output).
    """
    import time

    import jax
    import concourse.mybir as mybir
    from concourse import bass2jax
    from concourse.bass2jax import _bass_exec_p, partition_id_tensor
    from jax.sharding import Mesh, NamedSharding, PartitionSpec

    bass2jax.install_neuronx_cc_hook()
    nc = _get_program()
    in_maps = _prepare_in_maps(inputs)
    n_cores = 8

    partition_name = nc.partition_id_tensor.name if nc.partition_id_tensor else None
    in_names, out_names, out_avals = [], [], []
    for alloc in nc.m.functions[0].allocations:
        if not isinstance(alloc, mybir.MemoryLocationSet):
            continue
        name = alloc.memorylocations[0].name
        if alloc.kind == "ExternalInput":
            if name != partition_name:
                in_names.append(name)
        elif alloc.kind == "ExternalOutput":
            out_names.append(name)
            out_avals.append(jax.core.ShapedArray(
                tuple(alloc.tensor_shape), mybir.dt.np(alloc.dtype)))
    n_params = len(in_names)
    all_names = list(in_names) + out_names
    if partition_name is not None:
        all_names.append(partition_name)

    def _body(*args):
        operands = list(args)
        if partition_name is not None:
            operands.append(partition_id_tensor())
        outs = _bass_exec_p.bind(
            *operands,
            out_avals=tuple(out_avals),
            in_names=tuple(all_names),
            out_names=tuple(out_names),
            lowering_input_output_aliases=(),
            sim_require_finite=True,
            sim_require_nnan=True,
            nc=nc,
        )
        return tuple(outs)

    devices = jax.devices()[:n_cores]
    mesh = Mesh(np.asarray(devices), ("core",))
    spec = NamedSharding(mesh, PartitionSpec("core"))
    from jax.experimental.shard_map import shard_map
    sharded = jax.jit(shard_map(
        _body, mesh=mesh,
        in_specs=(PartitionSpec("core"),) * (n_params + len(out_names)),
        out_specs=(PartitionSpec("core"),) * len(out_names),
        check_rep=False), keep_unused=True)

    concat_in = [
        jax.device_put(
            np.concatenate([np.asarray(in_maps[c][nm]) for c in range(n_cores)],
                           axis=0), spec)
        for nm in in_names]
    concat_zeros = [
        jax.device_put(
            np.zeros((n_cores * a.shape[0], *a.shape[1:]), a.dtype), spec)
        for a in out_avals]
    out = sharded(*concat_in, *concat_zeros)
    jax.block_until_ready(out)
    t0 = time.perf_counter()
    for _ in range(iters):
        out = sharded(*concat_in, *concat_zeros)
    jax.block_until_ready(out)
    t1 = time.perf_counter()

    ys = np.asarray(out[out_names.index("y")]).reshape(n_cores, S, DIM)
    full = np.empty((B, S, DIM), np.float32)
    for b in range(B):
        full[b] = ys[2 * b] + ys[2 * b + 1]
    return (t1 - t0) / iters, full
